# revision 24
# baseline (speedup 1.0000x reference)
"""Trainium2 Bass kernel for AdvancedGATModel (2x GATv2 + skips + MLP).

Strategy (8 NeuronCores, node/dst-sharded, bf16 edge pipeline):
  - Nodes sharded 12500/core; edges sharded by destination. Per core, edges
    are bucketed by (dst-group g of 128 nodes, src-quarter q of the padded
    global node space), each (g,q) segment padded to whole 128-edge chunks,
    with an identical chunk schedule on all cores (one SPMD program).
  - Per layer: node GEMMs produce xl/xr/skip in one fused 384-col bf16
    matmul per 128-node tile; xl is AllGathered (bf16) to all cores.
  - Edge features xl[src] are fetched by a hybrid of the two SWDGE gather
    mechanisms, which drain through different DMA resources and overlap:
    quarters 0-1 use per-chunk indirect_dma_start (128 rows/instr, spread
    over 16 SDMA engines, Pool-issue-bound) and quarters 2-3 use bulk
    dma_gather (thousands of 256B rows per instr via int16 quarter-relative
    indices, single-engine-bandwidth-bound).
  - Segment softmax / scatter-add are exact one-hot matmuls (bf16):
      S[e,n] = (dst_in_group[e] == n)        (DVE is_equal vs bf16 iota row)
      z      = (S^T as lhsT) @ xr  +  I @ gx[src]     (fp32 PSUM)
      logits = reduce(Prelu(z, alpha=0.2) * att)      (ACT Prelu + DVE)
      numer/denom accumulated per group: (S as lhsT) @ [exp(l)*gx | exp(l)]
    exp() is applied without max-subtraction: any per-segment constant
    cancels in softmax and logits are O(1) so fp32 exp is well behaved.
"""

import os
import numpy as np
import ml_dtypes

import concourse.bass as bass
import concourse.mybir as mybir
import concourse.tile as tile
from concourse import bacc
from concourse import library_config
from concourse.bass_utils import run_bass_kernel_spmd

F32 = mybir.dt.float32
BF16 = mybir.dt.bfloat16
I16 = mybir.dt.int16
P = 128

CFG = dict(
    N=100000, E=1600000, FIN=128, HID=128, HEADS=4, OUT=64,
    NCORES=8, NEG=0.2, NQ=4, SG=3, GXF32=0, HYBQ=2,
)

_BUILD_CACHE = {}
LAST = {}  # debug/test hooks: results, exec_time_ns


# ---------------------------------------------------------------- host side

def _preprocess(cfg, edge_index):
    """Sort/shard/pad edges; build gather-index and dst tables per core.

    Edges per core bucketed by (g=dst_local//128, q=src_row//QROWS), each
    (g,q) segment padded to whole 128-slot chunks (same count on all cores).
    Returns the chunk schedule and per-core tables:
      cpgq [GROUPS, NQ]  chunks per (g,q)
      idx16 [NCORES, 128, TC*8] int16: slot i -> partitions {i%16+16k},
            col i//16 (SWDGE dma_gather wrapped layout), quarter-relative row
      dloc  [NCORES, 128, TC] f32: slot (c*128+p) -> local dst in group
    """
    N = cfg["N"]; NCORES = cfg["NCORES"]; NQ = cfg["NQ"]
    NLOC = N // NCORES
    GROUPS = (NLOC + P - 1) // P
    NLOC_PAD = GROUPS * P
    QROWS = NCORES * NLOC_PAD // NQ

    # self-loops are handled by a separate local per-group pass (no gather)
    src = np.asarray(edge_index[0], np.int64)
    dst = np.asarray(edge_index[1], np.int64)
    keep = src != dst
    src, dst = src[keep], dst[keep]

    scor = src // NLOC
    row = scor * NLOC_PAD + (src - scor * NLOC)     # padded global row
    q_of = row // QROWS
    core = dst // NLOC

    NGQ = GROUPS * NQ
    counts = np.zeros((NCORES, NGQ), np.int64)
    percore = []
    for m in range(NCORES):
        sel = np.nonzero(core == m)[0]
        ld = dst[sel] - m * NLOC
        r = row[sel]
        qq = q_of[sel]
        gq = (ld // P) * NQ + qq
        order = np.argsort(gq, kind="stable")
        percore.append((r[order], ld[order], gq[order]))
        counts[m] = np.bincount(gq, minlength=NGQ)

    cpgq = (counts.max(axis=0) + P - 1) // P        # [NGQ]
    chunk_base = np.zeros(NGQ, np.int64)
    np.cumsum(cpgq[:-1], out=chunk_base[1:])
    TC = int(cpgq.sum())
    NSLOT = TC * P

    idx16 = np.zeros((NCORES, 128, TC * 8), np.int16)
    gidx32 = np.zeros((NCORES, NSLOT), np.int32)
    dloc = np.full((NCORES, NSLOT), 200.0, np.float32)
    for m in range(NCORES):
        r, ld, gq = percore[m]
        off = np.zeros(NGQ, np.int64)
        np.cumsum(counts[m][:-1], out=off[1:])
        pos = np.arange(len(r)) - off[gq]
        slot = chunk_base[gq] * P + pos
        qrel = (r - (r // QROWS) * QROWS).astype(np.int16)
        iflat = np.zeros(NSLOT, np.int16)           # pad -> row 0 of quarter
        iflat[slot] = qrel
        blk = iflat.reshape(TC * 8, 16).T           # [16, TC*8]
        idx16[m] = np.tile(blk, (8, 1))
        g32 = np.zeros(NSLOT, np.int32)
        g32[slot] = r.astype(np.int32)
        gidx32[m] = g32
        dloc[m, slot] = (ld - (gq // NQ) * P).astype(np.float32)

    dloc_sb = np.ascontiguousarray(
        dloc.reshape(NCORES, TC, P).transpose(0, 2, 1))
    gidx32_sb = np.ascontiguousarray(
        gidx32.reshape(NCORES, TC, P).transpose(0, 2, 1))
    sched = dict(cpgq=cpgq.reshape(GROUPS, NQ), chunk_base=chunk_base,
                 TC=TC, NLOC=NLOC, GROUPS=GROUPS, NLOC_PAD=NLOC_PAD,
                 QROWS=QROWS)
    return sched, idx16, dloc_sb, gidx32_sb


# ------------------------------------------------------------- device build

def build_program(cfg, sched, debug_taps=()):
    FIN = cfg["FIN"]; HID = cfg["HID"]
    HEADS = cfg["HEADS"]; OUT = cfg["OUT"]; NCORES = cfg["NCORES"]
    NQ = cfg["NQ"]; SG = cfg["SG"]
    NLOC_PAD = sched["NLOC_PAD"]; GROUPS = sched["GROUPS"]
    QROWS = sched["QROWS"]; TC = sched["TC"]
    cpgq = sched["cpgq"]; chunk_base = sched["chunk_base"]
    C1 = HID // HEADS
    W1 = HID + HEADS      # scatter rhs width, layer 1
    W2 = HID + 1          # layer 2
    GBMAX = int(cpgq.max())
    GXF32 = bool(cfg.get("GXF32"))
    GXDT = F32 if GXF32 else BF16
    AF = mybir.ActivationFunctionType
    OP = mybir.AluOpType

    sgs = [list(range(s, min(s + SG, GROUPS))) for s in range(0, GROUPS, SG)]
    MAXSGC = max(int(cpgq[sg].sum()) for sg in sgs)

    nc = bacc.Bacc("TRN2", target_bir_lowering=False, debug=False,
                   num_devices=NCORES, num_swdge_queues=4)

    # ---- I/O
    xT_d = nc.dram_tensor("xT", [P, NLOC_PAD], BF16, kind="ExternalInput").ap()
    idx_d = nc.dram_tensor("idx16", [P, TC * 8], I16, kind="ExternalInput").ap()
    dloc_d = nc.dram_tensor("dloc", [P, TC], F32, kind="ExternalInput").ap()
    gi32_d = nc.dram_tensor("gidx32", [P, TC], mybir.dt.int32,
                            kind="ExternalInput").ap()
    out_d = nc.dram_tensor("out", [NLOC_PAD, OUT], F32, kind="ExternalOutput").ap()

    const_names = [
        ("iota", [P, P], BF16), ("identb", [P, P], BF16),
        ("identg", [P, P], GXDT),
        ("att1b", [P, GBMAX * P], BF16), ("att2b", [P, GBMAX * P], BF16),
        ("w1all", [FIN, 3 * HID], BF16), ("w2all", [HID, 3 * HID], BF16),
        ("mlp1", [HID, HID], BF16), ("mlp2", [HID, OUT], BF16),
        ("b1all", [P, 3 * HID], F32), ("b2all", [P, 3 * HID], F32),
        ("m1bb", [P, HID], F32), ("m2bb", [P, OUT], F32),
    ]
    const_d = {n: nc.dram_tensor(n, s, d, kind="ExternalInput").ap()
               for n, s, d in const_names}

    with tile.TileContext(nc) as tc:
        with (
            tc.tile_pool(name="constp", bufs=1) as constp,
            tc.tile_pool(name="dramp", bufs=1, space="DRAM") as dramp,
            tc.tile_pool(name="sbp", bufs=4) as sbp,
            tc.tile_pool(name="sbs", bufs=2) as sbs,
            tc.tile_pool(name="gatherp", bufs=2) as gatherp,
            tc.tile_pool(name="psZ", bufs=2, space="PSUM") as psZ,
            tc.tile_pool(name="psT", bufs=2, space="PSUM") as psT,
            tc.tile_pool(name="psB", bufs=2, space="PSUM") as psB,
            tc.tile_pool(name="psC", bufs=1, space="PSUM") as psC,
        ):
            # ---- DRAM intermediates (bf16)
            xl1_loc = dramp.tile([NLOC_PAD, HID], GXDT, tag="xl1_loc")
            xr1_loc = dramp.tile([NLOC_PAD, HID], BF16, tag="xr1_loc")
            sk1_loc = dramp.tile([NLOC_PAD, HID], BF16, tag="sk1_loc")
            xl1_full = dramp.tile([NCORES * NLOC_PAD, HID], GXDT,
                                  tag="xl1_full", addr_space="Shared")
            xl2_loc = dramp.tile([NLOC_PAD, HID], GXDT, tag="xl2_loc")
            xr2_loc = dramp.tile([NLOC_PAD, HID], BF16, tag="xr2_loc")
            sk2_loc = dramp.tile([NLOC_PAD, HID], BF16, tag="sk2_loc")
            xl2_full = dramp.tile([NCORES * NLOC_PAD, HID], GXDT,
                                  tag="xl2_full", addr_space="Shared")

            nc.gpsimd.load_library(library_config.mlp)

            # ---- resident constants / tables
            ct = {}
            for n, s, d in const_names:
                t = constp.tile(s, d, tag=n)
                nc.sync.dma_start(out=t[:], in_=const_d[n])
                ct[n] = t
            idx_t = constp.tile([P, TC * 8], I16, tag="idx16")
            nc.sync.dma_start(out=idx_t[:], in_=idx_d)
            gi32_t = constp.tile([P, TC], mybir.dt.int32, tag="gidx32")
            nc.sync.dma_start(out=gi32_t[:], in_=gi32_d)
            dloc_t = constp.tile([P, TC], F32, tag="dloc")
            nc.sync.dma_start(out=dloc_t[:], in_=dloc_d)

            # ================= phase A: layer-1 node GEMMs =================
            for g in range(GROUPS):
                xT_t = sbp.tile([P, P], BF16, tag="xTt")
                nc.sync.dma_start(out=xT_t[:], in_=xT_d[:, g * P:(g + 1) * P])
                pg = psC.tile([P, 3 * HID], F32, tag="gemm")
                nc.tensor.matmul(out=pg[:], lhsT=xT_t[:], rhs=ct["w1all"][:],
                                 start=True, stop=True)
                o = sbp.tile([P, 3 * HID], BF16, tag="gemmout")
                nc.vector.tensor_tensor(out=o[:], in0=pg[:], in1=ct["b1all"][:],
                                        op=OP.add)
                r0, r1 = g * P, (g + 1) * P
                if GXF32:
                    og = sbp.tile([P, HID], F32, tag="gemmoutg")
                    nc.vector.tensor_tensor(out=og[:], in0=pg[:, 0:HID],
                                            in1=ct["b1all"][:, 0:HID],
                                            op=OP.add)
                    nc.sync.dma_start(out=xl1_loc[r0:r1, :], in_=og[:])
                else:
                    nc.sync.dma_start(out=xl1_loc[r0:r1, :], in_=o[:, 0:HID])
                nc.sync.dma_start(out=xr1_loc[r0:r1, :], in_=o[:, HID:2 * HID])
                nc.sync.dma_start(out=sk1_loc[r0:r1, :], in_=o[:, 2 * HID:3 * HID])

            _agreps = int(os.environ.get("GAT_REPEAT_AG", "1"))
            nc.gpsimd.collective_compute(
                "AllGather", OP.bypass,
                replica_groups=[list(range(NCORES))],
                ins=[xl1_loc[:].opt()], outs=[xl1_full[:].opt()],
            )
            if _agreps > 1:
                # probe-only: extra AllGathers into a scratch Shared tensor
                # (Shared DRAM allows a single writer per tensor)
                for _rep in range(_agreps - 1):
                    dmy = dramp.tile([NCORES * NLOC_PAD, HID], GXDT,
                                     tag=f"xl1_dmy{_rep}", addr_space="Shared")
                    nc.gpsimd.collective_compute(
                        "AllGather", OP.bypass,
                        replica_groups=[list(range(NCORES))],
                        ins=[xl1_loc[:].opt()], outs=[dmy[:].opt()],
                    )

            # ================= edge pass (shared for both layers) ==========
            def edge_pass(layer, H, Ci, W, xl_full, xl_loc, xr_loc, attb,
                          epilogue):
                for sg in sgs:
                    sgc0 = int(chunk_base[sg[0] * NQ])          # first chunk
                    sgnc = int(cpgq[sg].sum())
                    gx = gatherp.tile([P, MAXSGC * P], GXDT, tag="gx")
                    HYBQ = int(cfg.get("HYBQ", 0))
                    for q in range(NQ):
                        idx_cols = []
                        for g in sg:
                            c0 = int(chunk_base[g * NQ + q])
                            ncn = int(cpgq[g][q])
                            if ncn:
                                idx_cols.append((c0, ncn))
                        for c0, ncn in idx_cols:
                            if q >= HYBQ:
                                # bulk SWDGE gather (single-engine BW-bound)
                                nc.gpsimd.dma_gather(
                                    gx[:, (c0 - sgc0) * P:(c0 - sgc0 + ncn) * P]
                                    .rearrange("p (c w) -> p c w", w=P),
                                    xl_full[q * QROWS:(q + 1) * QROWS, :],
                                    idx_t[:, c0 * 8:(c0 + ncn) * 8],
                                    ncn * P, ncn * P, P,
                                    single_packet=False, queue_num=q,
                                )
                            else:
                                # per-chunk indirect (16-engine, issue-bound):
                                # runs concurrently with the bulk gathers
                                for c in range(c0, c0 + ncn):
                                    nc.gpsimd.indirect_dma_start(
                                        out=gx[:, (c - sgc0) * P:
                                               (c - sgc0 + 1) * P],
                                        out_offset=None,
                                        in_=xl_full[:, :],
                                        in_offset=bass.IndirectOffsetOnAxis(
                                            ap=gi32_t[:, c:c + 1], axis=0),
                                    )
                    for g in sg:
                        xr_t = sbp.tile([P, HID], BF16, tag="xr")
                        nc.sync.dma_start(out=xr_t[:],
                                          in_=xr_loc[g * P:(g + 1) * P, :])
                        gout = psB.tile([P, W], F32, tag="gout")
                        nch = int(cpgq[g].sum())
                        # self-loop contribution: z = xl[n]+xr[n], no gather;
                        # the identity matmul opens the gout accumulation
                        xls = sbp.tile([P, HID], GXDT, tag="xls")
                        nc.sync.dma_start(out=xls[:],
                                          in_=xl_loc[g * P:(g + 1) * P, :])
                        zs = sbp.tile([P, HID], F32, tag="zs")
                        nc.vector.tensor_tensor(out=zs[:], in0=xls[:],
                                                in1=xr_t[:], op=OP.add)
                        zsl = sbp.tile([P, HID], BF16, tag="zsl")
                        nc.scalar.activation(out=zsl[:], in_=zs[:],
                                             func=AF.Prelu, alpha=0.2)
                        lms = sbp.tile([P, HID], BF16, tag="lms")
                        nc.vector.tensor_tensor(out=lms[:], in0=zsl[:],
                                                in1=attb[:, :HID], op=OP.mult)
                        l4s = sbp.tile([P, H], F32, tag="l4s")
                        nc.vector.tensor_reduce(
                            out=l4s[:].rearrange("p (c h) -> p c h", h=H),
                            in_=lms[:].rearrange("p (c h i) -> p c h i",
                                                 h=H, i=Ci),
                            axis=mybir.AxisListType.X, op=OP.add)
                        rhs_s = sbp.tile([P, W], BF16, tag="rhss")
                        nc.scalar.activation(out=rhs_s[:, HID:W], in_=l4s[:],
                                             func=AF.Exp)
                        nc.vector.tensor_tensor(
                            out=rhs_s[:, 0:HID].rearrange(
                                "p (h i) -> p h i", i=Ci),
                            in0=xls[:].rearrange("p (h i) -> p h i", i=Ci),
                            in1=rhs_s[:, HID:W][:, :, None]
                            .to_broadcast([P, H, Ci]),
                            op=OP.mult)
                        nc.tensor.matmul(out=gout[:, :W], lhsT=ct["identb"][:],
                                         rhs=rhs_s[:], start=True,
                                         stop=(nch == 0),
                                         skip_group_check=True)
                        done = 0
                        for q in range(NQ):
                            c0 = int(chunk_base[g * NQ + q])
                            ncn = int(cpgq[g][q])
                            if not ncn:
                                continue
                            # one batch per (g,q) segment
                            nb = ncn
                            S4 = sbs.tile([P, GBMAX * P], BF16, tag="S4")
                            for j in range(nb):
                                nc.vector.tensor_scalar(
                                    out=S4[:, j * P:(j + 1) * P],
                                    in0=ct["iota"][:],
                                    scalar1=dloc_t[:, c0 + j:c0 + j + 1],
                                    scalar2=None, op0=OP.is_equal)
                            STp = psT.tile([P, GBMAX * P], BF16, tag="STp")
                            for j in range(nb):
                                nc.tensor.transpose(
                                    out=STp[:, j * P:(j + 1) * P],
                                    in_=S4[:, j * P:(j + 1) * P],
                                    identity=ct["identb"][:])
                            ST4 = sbs.tile([P, GBMAX * P], BF16, tag="ST4")
                            nc.vector.tensor_copy(out=ST4[:, :nb * P],
                                                  in_=STp[:, :nb * P])
                            gcol = c0 - sgc0
                            zl = sbs.tile([P, GBMAX * P], BF16, tag="zl")
                            # z in 4-chunk PSUM sub-batches (one 2KB bank
                            # each): gx copy opens the zero region, per-chunk
                            # S^T@xr accumulates, Prelu drains to zl
                            for j0 in range(0, nb, 4):
                                j1 = min(j0 + 4, nb)
                                zp = psZ.tile([P, 4 * P], F32, tag="zp")
                                nc.tensor.matmul(
                                    out=zp[:, 0:(j1 - j0) * P],
                                    lhsT=ct["identg"][:],
                                    rhs=gx[:, (gcol + j0) * P:(gcol + j1) * P],
                                    start=True, stop=False)
                                for j in range(j0, j1):
                                    nc.tensor.matmul(
                                        out=zp[:, (j - j0) * P:(j - j0 + 1) * P],
                                        lhsT=ST4[:, j * P:(j + 1) * P],
                                        rhs=xr_t[:],
                                        start=False, stop=(j == j1 - 1))
                                nc.scalar.activation(
                                    out=zl[:, j0 * P:j1 * P],
                                    in_=zp[:, 0:(j1 - j0) * P],
                                    func=AF.Prelu, alpha=0.2)
                            lm = sbs.tile([P, GBMAX * P], BF16, tag="lm")
                            nc.vector.tensor_tensor(out=lm[:, :nb * P],
                                                    in0=zl[:, :nb * P],
                                                    in1=attb[:, :nb * P],
                                                    op=OP.mult)
                            l4 = sbs.tile([P, GBMAX * H], BF16, tag="l4")
                            with nc.allow_low_precision(
                                    reason="bf16 logits, 2e-2 budget"):
                                nc.vector.tensor_reduce(
                                    out=l4[:, :nb * H].rearrange(
                                        "p (c h) -> p c h", h=H),
                                    in_=lm[:, :nb * P].rearrange(
                                        "p (c h i) -> p c h i", h=H, i=Ci),
                                    axis=mybir.AxisListType.X, op=OP.add)
                            rhs4 = sbs.tile([P, GBMAX * W], BF16, tag="rhs4")
                            rview = rhs4[:, :nb * W].rearrange(
                                "p (c w) -> p c w", w=W)
                            a_view = rview[:, :, HID:W]
                            nc.scalar.activation(
                                out=a_view,
                                in_=l4[:, :nb * H].rearrange(
                                    "p (c h) -> p c h", h=H),
                                func=AF.Exp)
                            nc.vector.tensor_tensor(
                                out=rview[:, :, 0:HID].rearrange(
                                    "p c (h i) -> p c h i", i=Ci),
                                in0=gx[:, gcol * P:(gcol + nb) * P].rearrange(
                                    "p (c w) -> p c w", w=P).rearrange(
                                    "p c (h i) -> p c h i", i=Ci),
                                in1=a_view.to_broadcast([P, nb, H, Ci]),
                                op=OP.mult)
                            for j in range(nb):
                                nc.tensor.matmul(
                                    out=gout[:, :W],
                                    lhsT=S4[:, j * P:(j + 1) * P],
                                    rhs=rhs4[:, j * W:(j + 1) * W],
                                    start=False,
                                    stop=(done + j == nch - 1),
                                    skip_group_check=True)
                            done += nb
                        epilogue(g, gout)

            # ---- layer-1 epilogue: h = relu(gat + skip); fused layer-2 GEMMs
            def epilogue1(g, gout):
                H = HEADS
                dmx = sbp.tile([P, H], F32, tag="dmx")
                nc.vector.tensor_scalar(out=dmx[:], in0=gout[:, HID:HID + H],
                                        scalar1=1e-30, scalar2=None, op0=OP.max)
                rden = sbp.tile([P, H], F32, tag="rden")
                nc.vector.reciprocal(out=rden[:], in_=dmx[:])
                gat = sbp.tile([P, HID], F32, tag="gat")
                nc.vector.tensor_tensor(
                    out=gat[:].rearrange("p (h i) -> p h i", i=C1),
                    in0=gout[:, 0:HID].rearrange("p (h i) -> p h i", i=C1),
                    in1=rden[:, :, None].to_broadcast([P, H, C1]),
                    op=OP.mult)
                sk = sbp.tile([P, HID], BF16, tag="sk")
                nc.sync.dma_start(out=sk[:], in_=sk1_loc[g * P:(g + 1) * P, :])
                hs = sbp.tile([P, HID], F32, tag="hsb")
                nc.vector.tensor_tensor(out=hs[:], in0=gat[:], in1=sk[:],
                                        op=OP.add)
                hr = sbp.tile([P, HID], BF16, tag="hr")
                nc.vector.tensor_scalar(out=hr[:], in0=hs[:], scalar1=0.0,
                                        scalar2=None, op0=OP.max)
                hTf = psT.tile([P, GBMAX * P], BF16, tag="STp")
                hTp = hTf[:, 0:P]
                nc.tensor.transpose(out=hTp, in_=hr[:],
                                    identity=ct["identb"][:])
                hT = sbp.tile([P, P], BF16, tag="hT")
                nc.vector.tensor_copy(out=hT[:], in_=hTp)
                pg = psC.tile([P, 3 * HID], F32, tag="gemm")
                nc.tensor.matmul(out=pg[:], lhsT=hT[:], rhs=ct["w2all"][:],
                                 start=True, stop=True)
                o = sbp.tile([P, 3 * HID], BF16, tag="gemmout")
                nc.vector.tensor_tensor(out=o[:], in0=pg[:], in1=ct["b2all"][:],
                                        op=OP.add)
                r0, r1 = g * P, (g + 1) * P
                if GXF32:
                    og = sbp.tile([P, HID], F32, tag="gemmoutg")
                    nc.vector.tensor_tensor(out=og[:], in0=pg[:, 0:HID],
                                            in1=ct["b2all"][:, 0:HID],
                                            op=OP.add)
                    nc.sync.dma_start(out=xl2_loc[r0:r1, :], in_=og[:])
                else:
                    nc.sync.dma_start(out=xl2_loc[r0:r1, :], in_=o[:, 0:HID])
                nc.sync.dma_start(out=xr2_loc[r0:r1, :], in_=o[:, HID:2 * HID])
                nc.sync.dma_start(out=sk2_loc[r0:r1, :], in_=o[:, 2 * HID:3 * HID])

            # ---- layer-2 epilogue: hf = gat2 + skip2; MLP head
            def epilogue2(g, gout):
                dmx = sbp.tile([P, 1], F32, tag="dmx2")
                nc.vector.tensor_scalar(out=dmx[:], in0=gout[:, HID:HID + 1],
                                        scalar1=1e-30, scalar2=None, op0=OP.max)
                rden = sbp.tile([P, 1], F32, tag="rden2")
                nc.vector.reciprocal(out=rden[:], in_=dmx[:])
                sk = sbp.tile([P, HID], BF16, tag="sk2")
                nc.sync.dma_start(out=sk[:], in_=sk2_loc[g * P:(g + 1) * P, :])
                hf = sbp.tile([P, HID], BF16, tag="hf")
                nc.vector.scalar_tensor_tensor(
                    out=hf[:], in0=gout[:, 0:HID], scalar=rden[:, 0:1],
                    in1=sk[:], op0=OP.mult, op1=OP.add)
                hTf = psT.tile([P, GBMAX * P], BF16, tag="STp")
                hTp = hTf[:, 0:P]
                nc.tensor.transpose(out=hTp, in_=hf[:],
                                    identity=ct["identb"][:])
                hT = sbp.tile([P, P], BF16, tag="hT")
                nc.vector.tensor_copy(out=hT[:], in_=hTp)
                pmf = psC.tile([P, 3 * HID], F32, tag="gemm")
                pm = pmf[:, 0:HID]
                nc.tensor.matmul(out=pm, lhsT=hT[:], rhs=ct["mlp1"][:],
                                 start=True, stop=True)
                mb = sbp.tile([P, HID], F32, tag="mb")
                nc.vector.tensor_tensor(out=mb[:], in0=pm, in1=ct["m1bb"][:],
                                        op=OP.add)
                mr = sbp.tile([P, HID], BF16, tag="mr")
                nc.vector.tensor_scalar(out=mr[:], in0=mb[:], scalar1=0.0,
                                        scalar2=None, op0=OP.max)
                mTf = psT.tile([P, GBMAX * P], BF16, tag="STp")
                mTp = mTf[:, 0:P]
                nc.tensor.transpose(out=mTp, in_=mr[:],
                                    identity=ct["identb"][:])
                mT = sbp.tile([P, P], BF16, tag="hT")
                nc.vector.tensor_copy(out=mT[:], in_=mTp)
                pof = psC.tile([P, 3 * HID], F32, tag="gemm")
                po = pof[:, 0:HID]
                nc.tensor.matmul(out=po[:, 0:OUT], lhsT=mT[:],
                                 rhs=ct["mlp2"][:], start=True, stop=True)
                ot = sbp.tile([P, OUT], F32, tag="ot")
                nc.vector.tensor_tensor(out=ot[:], in0=po[:, 0:OUT],
                                        in1=ct["m2bb"][:], op=OP.add)
                nc.sync.dma_start(out=out_d[g * P:(g + 1) * P, :], in_=ot[:])

            for _rep in range(int(os.environ.get("GAT_REPEAT_EDGE1", "1"))):
                edge_pass(1, HEADS, C1, W1, xl1_full, xl1_loc, xr1_loc,
                          ct["att1b"], epilogue1)
            nc.gpsimd.collective_compute(
                "AllGather", OP.bypass,
                replica_groups=[list(range(NCORES))],
                ins=[xl2_loc[:].opt()], outs=[xl2_full[:].opt()],
            )
            edge_pass(2, 1, HID, W2, xl2_full, xl2_loc, xr2_loc,
                      ct["att2b"], epilogue2)

            # optional debug taps: copy internal DRAM tiles to outputs
            taps = {"xl1_loc": xl1_loc, "xr1_loc": xr1_loc, "sk1_loc": sk1_loc,
                    "xl1_full": xl1_full, "xl2_loc": xl2_loc,
                    "xr2_loc": xr2_loc, "sk2_loc": sk2_loc,
                    "xl2_full": xl2_full}
            for tap in debug_taps:
                if tap not in taps:
                    continue
                t = taps[tap]
                d = nc.dram_tensor(f"dbg_{tap}", list(t.shape), BF16,
                                   kind="ExternalOutput").ap()
                nc.sync.dma_start(out=d, in_=t[:])

    nc.compile()
    return nc


# ------------------------------------------------------------------ driver

def _make_inmaps(cfg, inputs, sched, idx16, dloc_sb, gidx32_sb):
    NCORES = cfg["NCORES"]; FIN = cfg["FIN"]; HID = cfg["HID"]
    HEADS = cfg["HEADS"]
    NLOC = sched["NLOC"]; NLOC_PAD = sched["NLOC_PAD"]
    GBMAX = int(sched["cpgq"].max())
    BF = ml_dtypes.bfloat16
    f = lambda k: np.asarray(inputs[k], np.float32)
    x = f("x")

    att1row = f("att1").reshape(1, HID)
    att2row = f("att2").reshape(1, HID)
    w1all = np.concatenate([f("Wl1"), f("Wr1"), f("skip1_w")], axis=1)
    w2all = np.concatenate([f("Wl2"), f("Wr2"), f("skip2_w")], axis=1)
    b1all = np.concatenate([f("bl1"), f("br1"), f("skip1_b") + f("b1")])
    b2all = np.concatenate([f("bl2"), f("br2"), f("skip2_b") + f("b2")])

    gxdt = np.float32 if cfg.get("GXF32") else BF
    consts = {
        "iota": np.tile(np.arange(P, dtype=np.float32)[None, :],
                        (P, 1)).astype(BF),
        "identb": np.eye(P, dtype=np.float32).astype(BF),
        "identg": np.eye(P, dtype=np.float32).astype(gxdt),
        "att1b": np.tile(att1row, (P, GBMAX)).astype(BF),
        "att2b": np.tile(att2row, (P, GBMAX)).astype(BF),
        "w1all": w1all.astype(BF), "w2all": w2all.astype(BF),
        "mlp1": f("mlp1_w").astype(BF), "mlp2": f("mlp2_w").astype(BF),
        "b1all": np.tile(b1all[None, :], (P, 1)).astype(np.float32),
        "b2all": np.tile(b2all[None, :], (P, 1)).astype(np.float32),
        "m1bb": np.tile(f("mlp1_b")[None, :], (P, 1)).astype(np.float32),
        "m2bb": np.tile(f("mlp2_b")[None, :], (P, 1)).astype(np.float32),
    }
    consts = {k: np.ascontiguousarray(v) for k, v in consts.items()}

    in_maps = []
    for m in range(NCORES):
        xm = x[m * NLOC:(m + 1) * NLOC]
        xT = np.zeros((FIN, NLOC_PAD), np.float32)
        xT[:, :NLOC] = xm.T
        im = {"xT": np.ascontiguousarray(xT.astype(BF)),
              "idx16": np.ascontiguousarray(idx16[m]),
              "gidx32": np.ascontiguousarray(gidx32_sb[m]),
              "dloc": np.ascontiguousarray(dloc_sb[m])}
        im.update(consts)
        in_maps.append(im)
    return in_maps


def _pjrt_runner(nc, in_maps, n_cores):
    """Build a persistent jitted runner (mirrors bass2jax.run_bass_via_pjrt)
    so executions can be timed steady-state with device-resident inputs."""
    import jax
    import concourse.mybir as mb
    from concourse import bass2jax
    from jax.sharding import Mesh, PartitionSpec
    from jax.experimental.shard_map import shard_map

    bass2jax.install_neuronx_cc_hook()
    partition_name = (nc.partition_id_tensor.name
                      if nc.partition_id_tensor else None)
    in_names, out_names, out_avals, zero_outs = [], [], [], []
    for alloc in nc.m.functions[0].allocations:
        if not isinstance(alloc, mb.MemoryLocationSet):
            continue
        name = alloc.memorylocations[0].name
        if alloc.kind == "ExternalInput":
            if name != partition_name:
                in_names.append(name)
        elif alloc.kind == "ExternalOutput":
            out_names.append(name)
            shape = tuple(alloc.tensor_shape)
            dtype = mb.dt.np(alloc.dtype)
            out_avals.append(jax.core.ShapedArray(shape, dtype))
            zero_outs.append(np.zeros(shape, dtype))
    n_params = len(in_names)
    n_outs = len(out_avals)
    all_in_names = list(in_names) + list(out_names)
    if partition_name is not None:
        all_in_names.append(partition_name)
    donate = tuple(range(n_params, n_params + n_outs))

    def _body(*args):
        operands = list(args)
        if partition_name is not None:
            operands.append(bass2jax.partition_id_tensor())
        outs = bass2jax._bass_exec_p.bind(
            *operands,
            out_avals=tuple(out_avals),
            in_names=tuple(all_in_names),
            out_names=tuple(out_names),
            lowering_input_output_aliases=(),
            sim_require_finite=True,
            sim_require_nnan=True,
            nc=nc,
        )
        return tuple(outs)

    devices = jax.devices()[:n_cores]
    mesh = Mesh(np.asarray(devices), ("core",))
    in_specs = (PartitionSpec("core"),) * (n_params + n_outs)
    out_specs = (PartitionSpec("core"),) * len(out_names)
    sharded = jax.jit(
        shard_map(_body, mesh=mesh, in_specs=in_specs, out_specs=out_specs,
                  check_rep=False),
        donate_argnums=donate, keep_unused=True)

    sharding = jax.sharding.NamedSharding(mesh, PartitionSpec("core"))
    concat_in = [
        jax.device_put(
            np.concatenate([np.asarray(in_maps[c][nm]) for c in range(n_cores)],
                           axis=0), sharding)
        for nm in in_names
    ]
    czero_shapes = [(n_cores * z.shape[0], *z.shape[1:]) for z in zero_outs]
    czero_dtypes = [z.dtype for z in zero_outs]

    def run():
        zeros = [jax.device_put(np.zeros(s, d), sharding)
                 for s, d in zip(czero_shapes, czero_dtypes)]
        for z in zeros:
            z.block_until_ready()
        import time
        t0 = time.perf_counter()
        outs = sharded(*concat_in, *zeros)
        for o in outs:
            o.block_until_ready()
        dt = time.perf_counter() - t0
        res = [
            {name: np.asarray(outs[i]).reshape(n_cores, *out_avals[i].shape)[c]
             for i, name in enumerate(out_names)}
            for c in range(n_cores)
        ]
        return res, dt

    return run


def kernel(**inputs):
    cfg = dict(CFG)
    if "GAT_GXF32" in os.environ:
        cfg["GXF32"] = int(os.environ["GAT_GXF32"])
    if "GAT_HYBQ" in os.environ:
        cfg["HYBQ"] = int(os.environ["GAT_HYBQ"])
    NCORES = cfg["NCORES"]
    sched, idx16, dloc_sb, gidx32_sb = _preprocess(
        cfg, np.asarray(inputs["edge_index"]))

    key = (tuple(int(c) for c in sched["cpgq"].ravel()),
           tuple(sorted((k, v) for k, v in cfg.items())))
    nc = _BUILD_CACHE.get(key)
    if nc is None:
        nc = build_program(cfg, sched)
        _BUILD_CACHE[key] = nc

    in_maps = _make_inmaps(cfg, inputs, sched, idx16, dloc_sb, gidx32_sb)
    NLOC = sched["NLOC"]
    bench_iters = int(os.environ.get("GAT_BENCH_ITERS", "0"))
    if bench_iters > 0:
        run = _pjrt_runner(nc, in_maps, NCORES)
        results, dt0 = run()          # warm-up (includes jit compile)
        times = []
        for _ in range(bench_iters):
            results, dt = run()
            times.append(dt)
        LAST["times"] = times
        LAST["exec_time_ns"] = int(min(times) * 1e9)
    else:
        res = run_bass_kernel_spmd(nc, in_maps, core_ids=list(range(NCORES)))
        results = res.results
        LAST["results"] = res
        LAST["exec_time_ns"] = res.exec_time_ns
    out = np.concatenate([results[m]["out"][:NLOC] for m in range(NCORES)],
                         axis=0)
    return out.astype(np.float32)
